# revision 9
# baseline (speedup 1.0000x reference)
"""VQ codebook-lookup kernel for Trainium2 (8 NeuronCores, data-parallel).

Problem: z (16,256,32,32) f32, embedding (8192,256) f32.
  zp = transpose(z) -> (N=16384, D=256) tokens
  d  = ||zp||^2 + ||e||^2 - 2 zp e^T ; idx = argmin_k d ; z_q = e[idx]
  out = zp + (z_q - zp)  (straight-through, fp32)
  q_loss = 1.25*mean((z_q-zp)^2)

Strategy
  - Shard z by batch across 8 cores (2 images = 2048 tokens per core);
    replicate the codebook.
  - Device (per core): C = zp @ (2e)^T via bf16 matmul (PE).  Scores are
    drained from PSUM by the Scalar engine (3/4, converting to bf16) and by
    the Vector engine (1/4, fused into the first max-fold).  A ttmax fold
    chain reduces 8192 scores/token to 1024 group-maxima (groups of 8,
    stride 1024); DVE max/max_index then yields the top-8 groups per token.
    Only the *candidate set* matters: top-8 groups x 8 codes = 64
    candidates per token.
  - Host: re-score the candidates replicating the reference's exact fp32
    arithmetic ((A+B) - 2M with IEEE fp32 rounding, first-index tie-break),
    which reproduces jnp.argmin's decisions bit-for-bit.  Rare tokens with
    uncertain candidate coverage fall back to an exact full-row re-score.

The argmin of the reference is numerically delicate: d sits on a ~256 base
so fp32 rounding quantizes scores into ~3e-5 buckets and ~1% of tokens are
decided by tie-breaking.  Replicating the fp32 rounding on the candidate
set is what makes the output match bit-for-bit.
"""

import numpy as np

K = 8192
D = 256
B = 16
HW = 1024            # 32*32
N = B * HW           # 16384 tokens
NCORES = 8
BPC = B // NCORES    # batches per core = 2
TPC = BPC * HW       # tokens per core = 2048
NBLK = TPC // 128    # 16 token blocks of 128
NTILE = 512          # matmul moving free dim
PSW = 1024           # psum tile width (2 banks)
ECH = 1024           # codebook SBUF chunk width
NG = 1024            # groups per token after fold chain
GRP = K // NG        # 8 codes per group, stride NG

_CACHE = {}


def _build_nc():
    import concourse.tile as tile
    from concourse import bacc, mybir

    f32 = mybir.dt.float32
    bf16 = mybir.dt.bfloat16
    u16 = mybir.dt.uint16
    MAX = mybir.AluOpType.max

    nc = bacc.Bacc("TRN2", target_bir_lowering=False, debug=False,
                   num_devices=NCORES)
    z_d = nc.dram_tensor("zb", (BPC, D, HW), bf16, kind="ExternalInput").ap()
    e_d = nc.dram_tensor("eb", (D, K), bf16, kind="ExternalInput").ap()
    idx_d = nc.dram_tensor("idx8", (NBLK, 128, 8), u16, kind="ExternalOutput").ap()
    val_d = nc.dram_tensor("val8", (NBLK, 128, 8), bf16, kind="ExternalOutput").ap()

    with tile.TileContext(nc) as tc:
        with tc.tile_pool(name="w", bufs=1) as wpool, \
             tc.tile_pool(name="c", bufs=4) as cpool, \
             tc.tile_pool(name="f", bufs=3) as fpool, \
             tc.tile_pool(name="o", bufs=4) as opool, \
             tc.tile_pool(name="ps", bufs=4, space="PSUM") as pspool:
            zsb = []
            for b in range(BPC):
                for kc in range(2):
                    t = wpool.tile([128, HW], bf16, tag=f"z{b}_{kc}",
                                   name=f"z{b}_{kc}")
                    nc.sync.dma_start(t[:], z_d[b, kc * 128:(kc + 1) * 128, :])
                    zsb.append(t)
            # codebook in 8 column-chunks per kc so compute starts after the
            # first chunk lands instead of after the full 4MB load
            NQ = K // ECH
            esb = [[None] * NQ for _ in range(2)]
            for q in range(NQ):
                for kc in range(2):
                    t = wpool.tile([128, ECH], bf16, tag=f"e{kc}_{q}",
                                   name=f"e{kc}_{q}")
                    nc.sync.dma_start(
                        t[:], e_d[kc * 128:(kc + 1) * 128,
                                  q * ECH:(q + 1) * ECH])
                    esb[kc][q] = t

            for tb in range(NBLK):
                b, tl = tb // 8, tb % 8
                lhs0 = zsb[2 * b][:, tl * 128:(tl + 1) * 128]
                lhs1 = zsb[2 * b + 1][:, tl * 128:(tl + 1) * 128]
                # cblk holds codes 0..6143 in bf16 (6 psum tiles via ACT)
                cblk = cpool.tile([128, 6 * PSW], bf16, name="cblk")
                t67 = []
                for pt in range(K // PSW):
                    ps = pspool.tile([128, PSW], f32, name="ps")
                    for h in range(PSW // NTILE):
                        nb = pt * (PSW // NTILE) + h
                        sl = slice(h * NTILE, (h + 1) * NTILE)
                        q, r = nb // 2, nb % 2
                        el = slice(r * NTILE, (r + 1) * NTILE)
                        nc.tensor.matmul(ps[:, sl], lhs0, esb[0][q][:, el],
                                         start=True, stop=False)
                        nc.tensor.matmul(ps[:, sl], lhs1, esb[1][q][:, el],
                                         start=False, stop=True)
                    if pt < 6:
                        nc.scalar.copy(cblk[:, pt * PSW:(pt + 1) * PSW], ps[:])
                    else:
                        t67.append(ps)
                # fold chain (all maxes are exact on bf16 values):
                # fa[c] = max over codes {c, c+2048}          (c < 2048)
                # ft[c] = max over codes {4096+c, 6144+c}     (c < 2048)
                # fb[c] = max over {c, c+2048, c+4096, c+6144}
                # fc[g] = max over {g + 1024*m, m=0..7}       (g < 1024)
                fa = fpool.tile([128, 2048], bf16, name="fa")
                nc.vector.tensor_tensor(fa[:], cblk[:, 0:2048],
                                        cblk[:, 2048:4096], op=MAX)
                ft = fpool.tile([128, 2048], bf16, name="ft")
                nc.vector.tensor_tensor(ft[:, 0:1024], cblk[:, 4096:5120],
                                        t67[0][:], op=MAX)
                nc.vector.tensor_tensor(ft[:, 1024:2048], cblk[:, 5120:6144],
                                        t67[1][:], op=MAX)
                fb = fpool.tile([128, 2048], bf16, name="fb")
                nc.vector.tensor_tensor(fb[:], fa[:], ft[:], op=MAX)
                fc = fpool.tile([128, NG], bf16, name="fc")
                nc.vector.tensor_tensor(fc[:], fb[:, 0:NG], fb[:, NG:2 * NG],
                                        op=MAX)
                v8 = opool.tile([128, 8], bf16, tag="v8", name="v8")
                i8 = opool.tile([128, 8], u16, tag="i8", name="i8")
                nc.vector.max(v8[:], fc[:])
                nc.vector.max_index(i8[:], v8[:], fc[:])
                nc.sync.dma_start(val_d[tb], v8[:])
                nc.sync.dma_start(idx_d[tb], i8[:])

    nc.compile()
    return nc


def _get_nc():
    if "nc" not in _CACHE:
        _CACHE["nc"] = _build_nc()
    return _CACHE["nc"]


def _run_device(z, e, trace=False):
    """Returns (group_idx (N,8) int64, group_val (N,8) float32)."""
    import ml_dtypes
    from concourse.bass_utils import run_bass_kernel_spmd

    nc = _get_nc()
    e2t = np.ascontiguousarray((2.0 * e).T).astype(ml_dtypes.bfloat16)
    zr = z.reshape(B, D, HW)
    in_maps = []
    for c in range(NCORES):
        zb = np.ascontiguousarray(zr[BPC * c:BPC * (c + 1)]).astype(
            ml_dtypes.bfloat16)
        in_maps.append({"zb": zb, "eb": e2t})
    res = run_bass_kernel_spmd(nc, in_maps, core_ids=list(range(NCORES)),
                               trace=trace)
    idx = np.concatenate(
        [res.results[c]["idx8"].reshape(TPC, 8).astype(np.int64)
         for c in range(NCORES)], axis=0)
    val = np.concatenate(
        [res.results[c]["val8"].reshape(TPC, 8).astype(np.float32)
         for c in range(NCORES)], axis=0)
    if trace:
        return idx, val, res
    return idx, val


def kernel(z, embedding):
    z = np.asarray(z, dtype=np.float32)
    e = np.asarray(embedding, dtype=np.float32)

    gidx, gval = _run_device(z, e)

    # Host: exact fp32 replication of the reference scoring on candidates.
    zp = np.ascontiguousarray(
        z.reshape(B, D, HW).transpose(0, 2, 1)).reshape(N, D)   # (N, D) f32
    z64 = zp.astype(np.float64)
    e64 = e.astype(np.float64)
    A32 = np.einsum("nd,nd->n", z64, z64).astype(np.float32)    # ||z_t||^2
    B32 = np.einsum("kd,kd->k", e64, e64).astype(np.float32)    # ||e_k||^2

    bad = (gidx < 0) | (gidx >= NG)
    gidx_s = np.where(bad, 0, gidx)
    # expand top-8 groups to 64 candidate codes (group g -> g + 1024*m)
    cand = (gidx_s[:, :, None] +
            NG * np.arange(GRP)[None, None, :]).reshape(N, 8 * GRP)

    NJ = 8 * GRP
    dj = np.empty((N, NJ), dtype=np.float32)
    CH = 2048
    for s in range(0, N, CH):
        sl = slice(s, s + CH)
        M32 = np.einsum("nd,njd->nj", z64[sl], e64[cand[sl]],
                        optimize=True).astype(np.float32)
        Tj = A32[sl, None] + B32[cand[sl]]       # fl32(A+B)
        dj[sl] = Tj - 2.0 * M32                  # fl32(T - C)
    dmin = dj.min(axis=1)
    idxf = np.where(dj == dmin[:, None], cand, K + 1).min(axis=1)

    # Fallback for tokens whose candidate set may not cover the tie zone:
    # top-8 group scores too crowded, or invalid device output.
    # Margin: 2 fp32 ulps @256 (6.1e-5) + bf16 matmul err (~2.5e-5*2)
    #         + bf16 score rounding (~3.1e-5*2) < 2.5e-4.
    risky = (gval[:, 0] - gval[:, 7] < 2.5e-4) | bad.any(axis=1)
    if risky.any():
        rt = np.nonzero(risky)[0]
        Mrow = (z64[rt] @ e64.T).astype(np.float32)     # (R, K)
        drow = (A32[rt, None] + B32[None, :]) - 2.0 * Mrow
        idxf[rt] = np.argmin(drow, axis=1)

    z_q = e[idxf]                                        # (N, D) f32 exact rows
    diff = z_q.astype(np.float64) - z64
    m32 = np.float32(np.mean(diff * diff))
    q_loss = np.float32(m32 + np.float32(0.25) * m32)

    # straight-through estimator, replicated in fp32: out = zp + (z_q - zp)
    st = zp + (z_q - zp)                                 # all f32, IEEE RN
    out = np.ascontiguousarray(
        st.reshape(B, HW, D).transpose(0, 2, 1)).reshape(B, D, 32, 32)
    return out, q_loss


# revision 11
# speedup vs baseline: 1.0039x; 1.0039x over previous
"""VQ codebook-lookup kernel for Trainium2 (8 NeuronCores, data-parallel).

Problem: z (16,256,32,32) f32, embedding (8192,256) f32.
  zp = transpose(z) -> (N=16384, D=256) tokens
  d  = ||zp||^2 + ||e||^2 - 2 zp e^T ; idx = argmin_k d ; z_q = e[idx]
  out = zp + (z_q - zp)  (straight-through, fp32)
  q_loss = 1.25*mean((z_q-zp)^2)

Strategy
  - Shard z by batch across 8 cores (2 images = 2048 tokens per core);
    replicate the codebook.
  - Device (per core): C = zp @ (2e)^T via bf16 matmul (PE).  Scores are
    drained from PSUM by the Scalar engine (6/8 tiles, converting to bf16)
    and by the Vector engine (2/8, fused into the first max-fold).  A ttmax
    fold chain reduces 8192 scores/token to 512 group-maxima (groups of 16,
    stride 512); DVE max/max_index then yields the top-8 groups per token.
    Only the *candidate set* matters.
  - Host: re-score the candidate groups (prefiltered by score margin)
    replicating the reference's exact fp32 arithmetic ((A+B) - 2M with
    IEEE fp32 rounding, first-index tie-break), which reproduces
    jnp.argmin's decisions bit-for-bit.  Rare tokens with uncertain
    candidate coverage fall back to an exact full-row re-score.

The argmin of the reference is numerically delicate: d sits on a ~256 base
so fp32 rounding quantizes scores into ~3e-5 buckets and ~1% of tokens are
decided by tie-breaking.  Replicating the fp32 rounding on the candidate
set is what makes the output match bit-for-bit.
"""

import numpy as np

K = 8192
D = 256
B = 16
HW = 1024            # 32*32
N = B * HW           # 16384 tokens
NCORES = 8
BPC = B // NCORES    # batches per core = 2
TPC = BPC * HW       # tokens per core = 2048
NBLK = TPC // 128    # 16 token blocks of 128
NTILE = 512          # matmul moving free dim
PSW = 1024           # psum tile width (2 banks)
ECH = 2048           # codebook SBUF chunk width
NG = 512             # groups per token after fold chain
GRP = K // NG        # 16 codes per group, stride NG

_CACHE = {}


def _build_nc():
    import concourse.tile as tile
    from concourse import bacc, mybir

    f32 = mybir.dt.float32
    bf16 = mybir.dt.bfloat16
    u16 = mybir.dt.uint16
    MAX = mybir.AluOpType.max

    nc = bacc.Bacc("TRN2", target_bir_lowering=False, debug=False,
                   num_devices=NCORES)
    z_d = nc.dram_tensor("zb", (BPC, D, HW), bf16, kind="ExternalInput").ap()
    e_d = nc.dram_tensor("eb", (D, K), bf16, kind="ExternalInput").ap()
    idx_d = nc.dram_tensor("idx8", (NBLK, 128, 8), u16, kind="ExternalOutput").ap()
    val_d = nc.dram_tensor("val8", (NBLK, 128, 8), bf16, kind="ExternalOutput").ap()

    with tile.TileContext(nc) as tc:
        with tc.tile_pool(name="w", bufs=1) as wpool, \
             tc.tile_pool(name="c", bufs=4) as cpool, \
             tc.tile_pool(name="f", bufs=3) as fpool, \
             tc.tile_pool(name="o", bufs=4) as opool, \
             tc.tile_pool(name="ps", bufs=4, space="PSUM") as pspool:
            # z: one merged DMA per batch; tile cols = kc*1024 + t
            zsb = []
            for b in range(BPC):
                t = wpool.tile([128, 2 * HW], bf16, tag=f"z{b}", name=f"z{b}")
                nc.sync.dma_start(
                    t[:].rearrange("p (c t) -> p c t", c=2),
                    z_d[b].rearrange("(c p) t -> p c t", c=2))
                zsb.append(t)
            # codebook in 4 column-chunks per kc so compute starts after the
            # first chunk lands instead of after the full 4MB load
            NQ = K // ECH
            esb = [[None] * NQ for _ in range(2)]
            for q in range(NQ):
                for kc in range(2):
                    t = wpool.tile([128, ECH], bf16, tag=f"e{kc}_{q}",
                                   name=f"e{kc}_{q}")
                    nc.sync.dma_start(
                        t[:], e_d[kc * 128:(kc + 1) * 128,
                                  q * ECH:(q + 1) * ECH])
                    esb[kc][q] = t

            for tb in range(NBLK):
                b, tl = tb // 8, tb % 8
                lhs0 = zsb[b][:, tl * 128:(tl + 1) * 128]
                lhs1 = zsb[b][:, HW + tl * 128:HW + (tl + 1) * 128]
                # psum tile pt holds codes [1024*pt, 1024*(pt+1)).
                # pt 0,1 are DVE-consumed (fused fold); pt 2..7 -> ACT -> cblk
                cblk = cpool.tile([128, 6 * PSW], bf16, name="cblk")
                t01 = []
                for pt in range(K // PSW):
                    ps = pspool.tile([128, PSW], f32, name="ps")
                    for h in range(PSW // NTILE):
                        nb = pt * (PSW // NTILE) + h
                        sl = slice(h * NTILE, (h + 1) * NTILE)
                        q, r = nb // 4, nb % 4
                        el = slice(r * NTILE, (r + 1) * NTILE)
                        nc.tensor.matmul(ps[:, sl], lhs0, esb[0][q][:, el],
                                         start=True, stop=False)
                        nc.tensor.matmul(ps[:, sl], lhs1, esb[1][q][:, el],
                                         start=False, stop=True)
                    if pt < 2:
                        t01.append(ps)
                    else:
                        nc.scalar.copy(cblk[:, (pt - 2) * PSW:(pt - 1) * PSW],
                                       ps[:])
                # fold chain to groups {c + 512*m, m=0..15} (exact bf16 maxes)
                ft = fpool.tile([128, 2048], bf16, name="ft")
                nc.vector.tensor_tensor(ft[:, 0:1024], cblk[:, 0:1024],
                                        t01[0][:], op=MAX)      # {c,2048+c}
                nc.vector.tensor_tensor(ft[:, 1024:2048], cblk[:, 1024:2048],
                                        t01[1][:], op=MAX)      # {1024+c,3072+c}
                fa = fpool.tile([128, 2048], bf16, name="fa")
                nc.vector.tensor_tensor(fa[:, 0:1024], cblk[:, 2048:3072],
                                        cblk[:, 4096:5120], op=MAX)
                nc.vector.tensor_tensor(fa[:, 1024:2048], cblk[:, 3072:4096],
                                        cblk[:, 5120:6144], op=MAX)
                g0 = fpool.tile([128, 1024], bf16, name="g0")
                nc.vector.tensor_tensor(g0[:], ft[:, 0:1024], ft[:, 1024:2048],
                                        op=MAX)  # {c,1024+c,2048+c,3072+c}
                fb = fpool.tile([128, 1024], bf16, name="fb")
                nc.vector.tensor_tensor(fb[:], fa[:, 0:1024], fa[:, 1024:2048],
                                        op=MAX)  # {4096+c,...,7168+c}
                fc = fpool.tile([128, 1024], bf16, name="fc")
                nc.vector.tensor_tensor(fc[:], g0[:], fb[:], op=MAX)
                fd = fpool.tile([128, NG], bf16, name="fd")
                nc.vector.tensor_tensor(fd[:], fc[:, 0:NG], fc[:, NG:2 * NG],
                                        op=MAX)  # groups of 16, stride 512
                v8 = opool.tile([128, 8], bf16, tag="v8", name="v8")
                i8 = opool.tile([128, 8], u16, tag="i8", name="i8")
                nc.vector.max(v8[:], fd[:])
                nc.vector.max_index(i8[:], v8[:], fd[:])
                nc.sync.dma_start(val_d[tb], v8[:])
                nc.sync.dma_start(idx_d[tb], i8[:])

    nc.compile()
    return nc


def _get_nc():
    if "nc" not in _CACHE:
        _CACHE["nc"] = _build_nc()
    return _CACHE["nc"]


def _run_device(z, e, trace=False):
    """Returns (group_idx (N,8) int64, group_val (N,8) float32)."""
    import ml_dtypes
    from concourse.bass_utils import run_bass_kernel_spmd

    nc = _get_nc()
    e2t = np.ascontiguousarray((2.0 * e).T).astype(ml_dtypes.bfloat16)
    zr = z.reshape(B, D, HW)
    in_maps = []
    for c in range(NCORES):
        zb = np.ascontiguousarray(zr[BPC * c:BPC * (c + 1)]).astype(
            ml_dtypes.bfloat16)
        in_maps.append({"zb": zb, "eb": e2t})
    res = run_bass_kernel_spmd(nc, in_maps, core_ids=list(range(NCORES)),
                               trace=trace)
    idx = np.concatenate(
        [res.results[c]["idx8"].reshape(TPC, 8).astype(np.int64)
         for c in range(NCORES)], axis=0)
    val = np.concatenate(
        [res.results[c]["val8"].reshape(TPC, 8).astype(np.float32)
         for c in range(NCORES)], axis=0)
    if trace:
        return idx, val, res
    return idx, val


def kernel(z, embedding):
    z = np.asarray(z, dtype=np.float32)
    e = np.asarray(embedding, dtype=np.float32)

    gidx, gval = _run_device(z, e)

    # Host: exact fp32 replication of the reference scoring on candidates.
    zp = np.ascontiguousarray(
        z.reshape(B, D, HW).transpose(0, 2, 1)).reshape(N, D)   # (N, D) f32
    z64 = zp.astype(np.float64)
    e64 = e.astype(np.float64)
    A32 = np.einsum("nd,nd->n", z64, z64).astype(np.float32)    # ||z_t||^2
    B32 = np.einsum("kd,kd->k", e64, e64).astype(np.float32)    # ||e_k||^2

    bad = (gidx < 0) | (gidx >= NG)
    gidx_s = np.where(bad, 0, gidx)

    # The winner's group-max is within (fp32 tie buckets + bf16 roundings +
    # bf16 matmul error) of the best group; only groups inside that margin
    # can contain the argmin.  Groups come sorted by descending score.
    MARGIN = 2.2e-4
    need = gval >= (gval[:, :1] - MARGIN)        # prefix mask, col 0 True
    nneed = need.sum(axis=1)

    idxf = np.zeros(N, dtype=np.int64)
    dmin = np.zeros(N, dtype=np.float32)
    dupw = np.zeros(N, dtype=bool)
    moff = NG * np.arange(GRP)
    for n in range(1, 9):
        tsel = np.nonzero(nneed == n)[0]
        if len(tsel) == 0:
            continue
        cand = (gidx_s[tsel, :n, None] + moff[None, None, :]).reshape(
            len(tsel), n * GRP)
        M32 = np.einsum("nd,njd->nj", z64[tsel], e64[cand],
                        optimize=True).astype(np.float32)
        Tj = A32[tsel, None] + B32[cand]         # fl32(A+B)
        dj = Tj - 2.0 * M32                      # fl32(T - C)
        dm = dj.min(axis=1)
        idxf[tsel] = np.where(dj == dm[:, None], cand, K + 1).min(axis=1)
        dmin[tsel] = dm

    # Fallback for tokens whose candidate set may not cover the tie zone:
    # top-8 group scores too crowded, or invalid device output.
    risky = (gval[:, 0] - gval[:, 7] < 2.5e-4) | bad.any(axis=1) | dupw
    if risky.any():
        rt = np.nonzero(risky)[0]
        Mrow = (z64[rt] @ e64.T).astype(np.float32)     # (R, K)
        drow = (A32[rt, None] + B32[None, :]) - 2.0 * Mrow
        idxf[rt] = np.argmin(drow, axis=1)

    z_q = e[idxf]                                        # (N, D) f32 exact rows
    diff = z_q.astype(np.float64) - z64
    m32 = np.float32(np.mean(diff * diff))
    q_loss = np.float32(m32 + np.float32(0.25) * m32)

    # straight-through estimator, replicated in fp32: out = zp + (z_q - zp)
    st = zp + (z_q - zp)                                 # all f32, IEEE RN
    out = np.ascontiguousarray(
        st.reshape(B, HW, D).transpose(0, 2, 1)).reshape(B, D, 32, 32)
    return out, q_loss


# revision 13
# speedup vs baseline: 1.0345x; 1.0305x over previous
"""VQ codebook-lookup kernel for Trainium2 (8 NeuronCores, data-parallel).

Problem: z (16,256,32,32) f32, embedding (8192,256) f32.
  zp = transpose(z) -> (N=16384, D=256) tokens
  d  = ||zp||^2 + ||e||^2 - 2 zp e^T ; idx = argmin_k d ; z_q = e[idx]
  out = zp + (z_q - zp)  (straight-through, fp32)
  q_loss = 1.25*mean((z_q-zp)^2)

Strategy
  - Shard z by batch across 8 cores (2 images = 2048 tokens per core);
    replicate the codebook.
  - Device (per core): C = zp @ (2e)^T via bf16 matmul (PE).  Scores are
    drained from PSUM by the Scalar engine (6/8 tiles, converting to bf16)
    and by the Vector engine (2/8, fused into the first max-fold).  A ttmax
    fold chain reduces 8192 scores/token to 512 group-maxima (groups of 16,
    stride 512); DVE max/max_index then yields the top-8 groups per token.
    Only the *candidate set* matters.
  - Host: re-score the candidate groups (prefiltered by score margin)
    replicating the reference's exact fp32 arithmetic ((A+B) - 2M with
    IEEE fp32 rounding, first-index tie-break), which reproduces
    jnp.argmin's decisions bit-for-bit.  Rare tokens with uncertain
    candidate coverage fall back to an exact full-row re-score.

The argmin of the reference is numerically delicate: d sits on a ~256 base
so fp32 rounding quantizes scores into ~3e-5 buckets and ~1% of tokens are
decided by tie-breaking.  Replicating the fp32 rounding on the candidate
set is what makes the output match bit-for-bit.
"""

import numpy as np

K = 8192
D = 256
B = 16
HW = 1024            # 32*32
N = B * HW           # 16384 tokens
NCORES = 8
BPC = B // NCORES    # batches per core = 2
TPC = BPC * HW       # tokens per core = 2048
NBLK = TPC // 128    # 16 token blocks of 128
NTILE = 512          # matmul moving free dim
PSW = 1024           # psum tile width (2 banks)
ECH = 2048           # codebook SBUF chunk width
NG = 512             # groups per token after fold chain
GRP = K // NG        # 16 codes per group, stride NG

_CACHE = {}


def _build_nc():
    import concourse.tile as tile
    from concourse import bacc, mybir

    f32 = mybir.dt.float32
    bf16 = mybir.dt.bfloat16
    u16 = mybir.dt.uint16
    MAX = mybir.AluOpType.max

    nc = bacc.Bacc("TRN2", target_bir_lowering=False, debug=False,
                   num_devices=NCORES)
    z_d = nc.dram_tensor("zb", (BPC, D, HW), bf16, kind="ExternalInput").ap()
    e_d = nc.dram_tensor("eb", (D, K), bf16, kind="ExternalInput").ap()
    idx_d = nc.dram_tensor("idx8", (NBLK, 128, 8), u16, kind="ExternalOutput").ap()
    val_d = nc.dram_tensor("val8", (NBLK, 128, 8), bf16, kind="ExternalOutput").ap()

    with tile.TileContext(nc) as tc:
        with tc.tile_pool(name="w", bufs=1) as wpool, \
             tc.tile_pool(name="c", bufs=4) as cpool, \
             tc.tile_pool(name="f", bufs=3) as fpool, \
             tc.tile_pool(name="o", bufs=4) as opool, \
             tc.tile_pool(name="ps", bufs=4, space="PSUM") as pspool:
            # z: one merged DMA per batch; tile cols = kc*1024 + t
            zsb = []
            for b in range(BPC):
                t = wpool.tile([128, 2 * HW], bf16, tag=f"z{b}", name=f"z{b}")
                nc.sync.dma_start(
                    t[:].rearrange("p (c t) -> p c t", c=2),
                    z_d[b].rearrange("(c p) t -> p c t", c=2))
                zsb.append(t)
            # codebook in 4 column-chunks per kc so compute starts after the
            # first chunk lands instead of after the full 4MB load
            NQ = K // ECH
            esb = [[None] * NQ for _ in range(2)]
            for q in range(NQ):
                for kc in range(2):
                    t = wpool.tile([128, ECH], bf16, tag=f"e{kc}_{q}",
                                   name=f"e{kc}_{q}")
                    nc.sync.dma_start(
                        t[:], e_d[kc * 128:(kc + 1) * 128,
                                  q * ECH:(q + 1) * ECH])
                    esb[kc][q] = t

            for tb in range(NBLK):
                b, tl = tb // 8, tb % 8
                lhs0 = zsb[b][:, tl * 128:(tl + 1) * 128]
                lhs1 = zsb[b][:, HW + tl * 128:HW + (tl + 1) * 128]
                # psum tile pt holds codes [1024*pt, 1024*(pt+1)).
                # pt 2,3 are DVE-consumed (fused fold, partnered with the
                # first-drained pt 0,1); the rest go ACT -> cblk at column
                # order [pt0|pt1|pt4|pt5|pt6|pt7]
                cblk = cpool.tile([128, 6 * PSW], bf16, name="cblk")
                ccol = {0: 0, 1: 1, 4: 2, 5: 3, 6: 4, 7: 5}
                t23 = []
                for pt in range(K // PSW):
                    ps = pspool.tile([128, PSW], f32, name="ps")
                    for h in range(PSW // NTILE):
                        nb = pt * (PSW // NTILE) + h
                        sl = slice(h * NTILE, (h + 1) * NTILE)
                        q, r = nb // 4, nb % 4
                        el = slice(r * NTILE, (r + 1) * NTILE)
                        nc.tensor.matmul(ps[:, sl], lhs0, esb[0][q][:, el],
                                         start=True, stop=False)
                        nc.tensor.matmul(ps[:, sl], lhs1, esb[1][q][:, el],
                                         start=False, stop=True)
                    if pt in (2, 3):
                        t23.append(ps)
                    else:
                        cc = ccol[pt]
                        nc.scalar.copy(cblk[:, cc * PSW:(cc + 1) * PSW], ps[:])
                # fold chain to groups {c + 512*m, m=0..15} (exact bf16
                # maxes; every pairing joins columns equal mod 512, so the
                # final groups are the full residue classes)
                ft = fpool.tile([128, 2048], bf16, name="ft")
                nc.vector.tensor_tensor(ft[:, 0:1024], cblk[:, 0:1024],
                                        t23[0][:], op=MAX)      # {c,2048+c}
                nc.vector.tensor_tensor(ft[:, 1024:2048], cblk[:, 1024:2048],
                                        t23[1][:], op=MAX)      # {1024+c,3072+c}
                fa = fpool.tile([128, 2048], bf16, name="fa")
                nc.vector.tensor_tensor(fa[:, 0:1024], cblk[:, 2048:3072],
                                        cblk[:, 4096:5120], op=MAX)
                nc.vector.tensor_tensor(fa[:, 1024:2048], cblk[:, 3072:4096],
                                        cblk[:, 5120:6144], op=MAX)
                g0 = fpool.tile([128, 1024], bf16, name="g0")
                nc.vector.tensor_tensor(g0[:], ft[:, 0:1024], ft[:, 1024:2048],
                                        op=MAX)  # {c,1024+c,2048+c,3072+c}
                fb = fpool.tile([128, 1024], bf16, name="fb")
                nc.vector.tensor_tensor(fb[:], fa[:, 0:1024], fa[:, 1024:2048],
                                        op=MAX)  # {4096+c,...,7168+c}
                fc = fpool.tile([128, 1024], bf16, name="fc")
                nc.vector.tensor_tensor(fc[:], g0[:], fb[:], op=MAX)
                fd = fpool.tile([128, NG], bf16, name="fd")
                nc.vector.tensor_tensor(fd[:], fc[:, 0:NG], fc[:, NG:2 * NG],
                                        op=MAX)  # groups of 16, stride 512
                v8 = opool.tile([128, 8], bf16, tag="v8", name="v8")
                i8 = opool.tile([128, 8], u16, tag="i8", name="i8")
                nc.vector.max(v8[:], fd[:])
                nc.vector.max_index(i8[:], v8[:], fd[:])
                nc.sync.dma_start(val_d[tb], v8[:])
                nc.sync.dma_start(idx_d[tb], i8[:])

    nc.compile()
    return nc


def _get_nc():
    if "nc" not in _CACHE:
        _CACHE["nc"] = _build_nc()
    return _CACHE["nc"]


def _run_device(z, e, trace=False):
    """Returns (group_idx (N,8) int64, group_val (N,8) float32)."""
    import ml_dtypes
    from concourse.bass_utils import run_bass_kernel_spmd

    nc = _get_nc()
    e2t = np.ascontiguousarray((2.0 * e).T).astype(ml_dtypes.bfloat16)
    zr = z.reshape(B, D, HW)
    in_maps = []
    for c in range(NCORES):
        zb = np.ascontiguousarray(zr[BPC * c:BPC * (c + 1)]).astype(
            ml_dtypes.bfloat16)
        in_maps.append({"zb": zb, "eb": e2t})
    res = run_bass_kernel_spmd(nc, in_maps, core_ids=list(range(NCORES)),
                               trace=trace)
    idx = np.concatenate(
        [res.results[c]["idx8"].reshape(TPC, 8).astype(np.int64)
         for c in range(NCORES)], axis=0)
    val = np.concatenate(
        [res.results[c]["val8"].reshape(TPC, 8).astype(np.float32)
         for c in range(NCORES)], axis=0)
    if trace:
        return idx, val, res
    return idx, val


def kernel(z, embedding):
    z = np.asarray(z, dtype=np.float32)
    e = np.asarray(embedding, dtype=np.float32)

    gidx, gval = _run_device(z, e)

    # Host: exact fp32 replication of the reference scoring on candidates.
    zp = np.ascontiguousarray(
        z.reshape(B, D, HW).transpose(0, 2, 1)).reshape(N, D)   # (N, D) f32
    z64 = zp.astype(np.float64)
    e64 = e.astype(np.float64)
    A32 = np.einsum("nd,nd->n", z64, z64).astype(np.float32)    # ||z_t||^2
    B32 = np.einsum("kd,kd->k", e64, e64).astype(np.float32)    # ||e_k||^2

    bad = (gidx < 0) | (gidx >= NG)
    gidx_s = np.where(bad, 0, gidx)

    # The winner's group-max is within (fp32 tie buckets + bf16 roundings +
    # bf16 matmul error) of the best group; only groups inside that margin
    # can contain the argmin.  Groups come sorted by descending score.
    MARGIN = 2.2e-4
    need = gval >= (gval[:, :1] - MARGIN)        # prefix mask, col 0 True
    nneed = need.sum(axis=1)

    idxf = np.zeros(N, dtype=np.int64)
    dmin = np.zeros(N, dtype=np.float32)
    dupw = np.zeros(N, dtype=bool)
    moff = NG * np.arange(GRP)
    for n in range(1, 9):
        tsel = np.nonzero(nneed == n)[0]
        if len(tsel) == 0:
            continue
        cand = (gidx_s[tsel, :n, None] + moff[None, None, :]).reshape(
            len(tsel), n * GRP)
        M32 = np.einsum("nd,njd->nj", z64[tsel], e64[cand],
                        optimize=True).astype(np.float32)
        Tj = A32[tsel, None] + B32[cand]         # fl32(A+B)
        dj = Tj - 2.0 * M32                      # fl32(T - C)
        dm = dj.min(axis=1)
        idxf[tsel] = np.where(dj == dm[:, None], cand, K + 1).min(axis=1)
        dmin[tsel] = dm

    # Fallback for tokens whose candidate set may not cover the tie zone:
    # top-8 group scores too crowded, or invalid device output.
    risky = (gval[:, 0] - gval[:, 7] < 2.5e-4) | bad.any(axis=1) | dupw
    if risky.any():
        rt = np.nonzero(risky)[0]
        Mrow = (z64[rt] @ e64.T).astype(np.float32)     # (R, K)
        drow = (A32[rt, None] + B32[None, :]) - 2.0 * Mrow
        idxf[rt] = np.argmin(drow, axis=1)

    z_q = e[idxf]                                        # (N, D) f32 exact rows
    diff = z_q.astype(np.float64) - z64
    m32 = np.float32(np.mean(diff * diff))
    q_loss = np.float32(m32 + np.float32(0.25) * m32)

    # straight-through estimator, replicated in fp32: out = zp + (z_q - zp)
    st = zp + (z_q - zp)                                 # all f32, IEEE RN
    out = np.ascontiguousarray(
        st.reshape(B, HW, D).transpose(0, 2, 1)).reshape(B, D, 32, 32)
    return out, q_loss


# revision 14
# speedup vs baseline: 1.0600x; 1.0246x over previous
"""VQ codebook-lookup kernel for Trainium2 (8 NeuronCores, data-parallel).

Problem: z (16,256,32,32) f32, embedding (8192,256) f32.
  zp = transpose(z) -> (N=16384, D=256) tokens
  d  = ||zp||^2 + ||e||^2 - 2 zp e^T ; idx = argmin_k d ; z_q = e[idx]
  out = zp + (z_q - zp)  (straight-through, fp32)
  q_loss = 1.25*mean((z_q-zp)^2)

Strategy
  - Shard z by batch across 8 cores (2 images = 2048 tokens per core);
    replicate the codebook.
  - Device (per core): C = zp @ (2e)^T via bf16 matmul (PE).  Scores are
    drained from PSUM by the Scalar engine (6/8 tiles, converting to bf16)
    and by the Vector engine (2/8, fused into the first max-fold).  A ttmax
    fold chain reduces 8192 scores/token to 512 group-maxima (groups of 16,
    stride 512); DVE max/max_index then yields the top-8 groups per token.
    Only the *candidate set* matters.
  - Host: re-score the candidate groups (prefiltered by score margin)
    replicating the reference's exact fp32 arithmetic ((A+B) - 2M with
    IEEE fp32 rounding, first-index tie-break), which reproduces
    jnp.argmin's decisions bit-for-bit.  Rare tokens with uncertain
    candidate coverage fall back to an exact full-row re-score.

The argmin of the reference is numerically delicate: d sits on a ~256 base
so fp32 rounding quantizes scores into ~3e-5 buckets and ~1% of tokens are
decided by tie-breaking.  Replicating the fp32 rounding on the candidate
set is what makes the output match bit-for-bit.
"""

import numpy as np

K = 8192
D = 256
B = 16
HW = 1024            # 32*32
N = B * HW           # 16384 tokens
NCORES = 8
BPC = B // NCORES    # batches per core = 2
TPC = BPC * HW       # tokens per core = 2048
NBLK = TPC // 128    # 16 token blocks of 128
NTILE = 512          # matmul moving free dim
PSW = 1024           # psum tile width (2 banks)
ECH = 2048           # codebook SBUF chunk width
NG = 512             # groups per token after fold chain
GRP = K // NG        # 16 codes per group, stride NG

_CACHE = {}


def _build_nc():
    import concourse.tile as tile
    from concourse import bacc, mybir

    f32 = mybir.dt.float32
    bf16 = mybir.dt.bfloat16
    u16 = mybir.dt.uint16
    MAX = mybir.AluOpType.max

    nc = bacc.Bacc("TRN2", target_bir_lowering=False, debug=False,
                   num_devices=NCORES)
    z_d = nc.dram_tensor("zb", (BPC, D, HW), bf16, kind="ExternalInput").ap()
    e_d = nc.dram_tensor("eb", (D, K), bf16, kind="ExternalInput").ap()
    idx_d = nc.dram_tensor("idx8", (NBLK, 128, 8), u16, kind="ExternalOutput").ap()
    val_d = nc.dram_tensor("val8", (NBLK, 128, 8), bf16, kind="ExternalOutput").ap()

    with tile.TileContext(nc) as tc:
        with tc.tile_pool(name="w", bufs=1) as wpool, \
             tc.tile_pool(name="c", bufs=4) as cpool, \
             tc.tile_pool(name="f", bufs=3) as fpool, \
             tc.tile_pool(name="o", bufs=4) as opool, \
             tc.tile_pool(name="ps", bufs=4, space="PSUM") as pspool:
            # z: one merged DMA per batch; tile cols = kc*1024 + t
            zsb = []
            for b in range(BPC):
                t = wpool.tile([128, 2 * HW], bf16, tag=f"z{b}", name=f"z{b}")
                nc.sync.dma_start(
                    t[:].rearrange("p (c t) -> p c t", c=2),
                    z_d[b].rearrange("(c p) t -> p c t", c=2))
                zsb.append(t)
            # codebook in 4 column-chunks per kc so compute starts after the
            # first chunk lands instead of after the full 4MB load
            NQ = K // ECH
            esb = [[None] * NQ for _ in range(2)]
            for q in range(NQ):
                for kc in range(2):
                    t = wpool.tile([128, ECH], bf16, tag=f"e{kc}_{q}",
                                   name=f"e{kc}_{q}")
                    nc.sync.dma_start(
                        t[:], e_d[kc * 128:(kc + 1) * 128,
                                  q * ECH:(q + 1) * ECH])
                    esb[kc][q] = t

            def finish_block(st):
                """Tail folds + top-8 for a block (deferred one block)."""
                tb, ft, fa = st
                g0 = fpool.tile([128, 1024], bf16, name="g0")
                nc.vector.tensor_tensor(g0[:], ft[:, 0:1024],
                                        ft[:, 1024:2048], op=MAX)
                fb = fpool.tile([128, 1024], bf16, name="fb")
                nc.vector.tensor_tensor(fb[:], fa[:, 0:1024],
                                        fa[:, 1024:2048], op=MAX)
                fc = fpool.tile([128, 1024], bf16, name="fc")
                nc.vector.tensor_tensor(fc[:], g0[:], fb[:], op=MAX)
                fd = fpool.tile([128, NG], bf16, name="fd")
                nc.vector.tensor_tensor(fd[:], fc[:, 0:NG],
                                        fc[:, NG:2 * NG], op=MAX)
                v8 = opool.tile([128, 8], bf16, tag="v8", name="v8")
                i8 = opool.tile([128, 8], u16, tag="i8", name="i8")
                nc.vector.max(v8[:], fd[:])
                nc.vector.max_index(i8[:], v8[:], fd[:])
                nc.sync.dma_start(val_d[tb], v8[:])
                nc.sync.dma_start(idx_d[tb], i8[:])

            pending = None
            for tb in range(NBLK):
                b, tl = tb // 8, tb % 8
                lhs0 = zsb[b][:, tl * 128:(tl + 1) * 128]
                lhs1 = zsb[b][:, HW + tl * 128:HW + (tl + 1) * 128]
                # psum tile pt holds codes [1024*pt, 1024*(pt+1)).
                # pt 2,3 are DVE-consumed (fused fold, partnered with the
                # first-drained pt 0,1); the rest go ACT -> cblk at column
                # order [pt0|pt1|pt4|pt5|pt6|pt7]
                cblk = cpool.tile([128, 6 * PSW], bf16, name="cblk")
                ccol = {0: 0, 1: 1, 4: 2, 5: 3, 6: 4, 7: 5}
                t23 = []
                for pt in range(K // PSW):
                    ps = pspool.tile([128, PSW], f32, name="ps")
                    for h in range(PSW // NTILE):
                        nb = pt * (PSW // NTILE) + h
                        sl = slice(h * NTILE, (h + 1) * NTILE)
                        q, r = nb // 4, nb % 4
                        el = slice(r * NTILE, (r + 1) * NTILE)
                        nc.tensor.matmul(ps[:, sl], lhs0, esb[0][q][:, el],
                                         start=True, stop=False)
                        nc.tensor.matmul(ps[:, sl], lhs1, esb[1][q][:, el],
                                         start=False, stop=True)
                    if pt in (2, 3):
                        t23.append(ps)
                    else:
                        cc = ccol[pt]
                        nc.scalar.copy(cblk[:, cc * PSW:(cc + 1) * PSW], ps[:])
                # fold chain to groups {c + 512*m, m=0..15} (exact bf16
                # maxes; every pairing joins columns equal mod 512, so the
                # final groups are the full residue classes)
                ft = fpool.tile([128, 2048], bf16, name="ft")
                nc.vector.tensor_tensor(ft[:, 0:1024], cblk[:, 0:1024],
                                        t23[0][:], op=MAX)      # {c,2048+c}
                nc.vector.tensor_tensor(ft[:, 1024:2048], cblk[:, 1024:2048],
                                        t23[1][:], op=MAX)      # {1024+c,3072+c}
                fa = fpool.tile([128, 2048], bf16, name="fa")
                nc.vector.tensor_tensor(fa[:, 0:1024], cblk[:, 2048:3072],
                                        cblk[:, 4096:5120], op=MAX)
                nc.vector.tensor_tensor(fa[:, 1024:2048], cblk[:, 3072:4096],
                                        cblk[:, 5120:6144], op=MAX)
                if pending is not None:
                    finish_block(pending)
                pending = (tb, ft, fa)
            finish_block(pending)

    nc.compile()
    return nc


def _get_nc():
    if "nc" not in _CACHE:
        _CACHE["nc"] = _build_nc()
    return _CACHE["nc"]


def _run_device(z, e, trace=False):
    """Returns (group_idx (N,8) int64, group_val (N,8) float32)."""
    import ml_dtypes
    from concourse.bass_utils import run_bass_kernel_spmd

    nc = _get_nc()
    e2t = np.ascontiguousarray((2.0 * e).T).astype(ml_dtypes.bfloat16)
    zr = z.reshape(B, D, HW)
    in_maps = []
    for c in range(NCORES):
        zb = np.ascontiguousarray(zr[BPC * c:BPC * (c + 1)]).astype(
            ml_dtypes.bfloat16)
        in_maps.append({"zb": zb, "eb": e2t})
    res = run_bass_kernel_spmd(nc, in_maps, core_ids=list(range(NCORES)),
                               trace=trace)
    idx = np.concatenate(
        [res.results[c]["idx8"].reshape(TPC, 8).astype(np.int64)
         for c in range(NCORES)], axis=0)
    val = np.concatenate(
        [res.results[c]["val8"].reshape(TPC, 8).astype(np.float32)
         for c in range(NCORES)], axis=0)
    if trace:
        return idx, val, res
    return idx, val


def kernel(z, embedding):
    z = np.asarray(z, dtype=np.float32)
    e = np.asarray(embedding, dtype=np.float32)

    gidx, gval = _run_device(z, e)

    # Host: exact fp32 replication of the reference scoring on candidates.
    zp = np.ascontiguousarray(
        z.reshape(B, D, HW).transpose(0, 2, 1)).reshape(N, D)   # (N, D) f32
    z64 = zp.astype(np.float64)
    e64 = e.astype(np.float64)
    A32 = np.einsum("nd,nd->n", z64, z64).astype(np.float32)    # ||z_t||^2
    B32 = np.einsum("kd,kd->k", e64, e64).astype(np.float32)    # ||e_k||^2

    bad = (gidx < 0) | (gidx >= NG)
    gidx_s = np.where(bad, 0, gidx)

    # The winner's group-max is within (fp32 tie buckets + bf16 roundings +
    # bf16 matmul error) of the best group; only groups inside that margin
    # can contain the argmin.  Groups come sorted by descending score.
    MARGIN = 2.2e-4
    need = gval >= (gval[:, :1] - MARGIN)        # prefix mask, col 0 True
    nneed = need.sum(axis=1)

    idxf = np.zeros(N, dtype=np.int64)
    dmin = np.zeros(N, dtype=np.float32)
    dupw = np.zeros(N, dtype=bool)
    moff = NG * np.arange(GRP)
    for n in range(1, 9):
        tsel = np.nonzero(nneed == n)[0]
        if len(tsel) == 0:
            continue
        cand = (gidx_s[tsel, :n, None] + moff[None, None, :]).reshape(
            len(tsel), n * GRP)
        M32 = np.einsum("nd,njd->nj", z64[tsel], e64[cand],
                        optimize=True).astype(np.float32)
        Tj = A32[tsel, None] + B32[cand]         # fl32(A+B)
        dj = Tj - 2.0 * M32                      # fl32(T - C)
        dm = dj.min(axis=1)
        idxf[tsel] = np.where(dj == dm[:, None], cand, K + 1).min(axis=1)
        dmin[tsel] = dm

    # Fallback for tokens whose candidate set may not cover the tie zone:
    # top-8 group scores too crowded, or invalid device output.
    risky = (gval[:, 0] - gval[:, 7] < 2.5e-4) | bad.any(axis=1) | dupw
    if risky.any():
        rt = np.nonzero(risky)[0]
        Mrow = (z64[rt] @ e64.T).astype(np.float32)     # (R, K)
        drow = (A32[rt, None] + B32[None, :]) - 2.0 * Mrow
        idxf[rt] = np.argmin(drow, axis=1)

    z_q = e[idxf]                                        # (N, D) f32 exact rows
    diff = z_q.astype(np.float64) - z64
    m32 = np.float32(np.mean(diff * diff))
    q_loss = np.float32(m32 + np.float32(0.25) * m32)

    # straight-through estimator, replicated in fp32: out = zp + (z_q - zp)
    st = zp + (z_q - zp)                                 # all f32, IEEE RN
    out = np.ascontiguousarray(
        st.reshape(B, HW, D).transpose(0, 2, 1)).reshape(B, D, 32, 32)
    return out, q_loss


# revision 15
# speedup vs baseline: 1.0659x; 1.0055x over previous
"""VQ codebook-lookup kernel for Trainium2 (8 NeuronCores, data-parallel).

Problem: z (16,256,32,32) f32, embedding (8192,256) f32.
  zp = transpose(z) -> (N=16384, D=256) tokens
  d  = ||zp||^2 + ||e||^2 - 2 zp e^T ; idx = argmin_k d ; z_q = e[idx]
  out = zp + (z_q - zp)  (straight-through, fp32)
  q_loss = 1.25*mean((z_q-zp)^2)

Strategy
  - Shard z by batch across 8 cores (2 images = 2048 tokens per core);
    replicate the codebook.
  - Device (per core): C = zp @ (2e)^T via bf16 matmul (PE).  Scores are
    drained from PSUM by the Scalar engine (6/8 tiles, converting to bf16)
    and by the Vector engine (2/8, fused into the first max-fold).  A ttmax
    fold chain reduces 8192 scores/token to 512 group-maxima (groups of 16,
    stride 512); DVE max/max_index then yields the top-8 groups per token.
    Only the *candidate set* matters.
  - Host: re-score the candidate groups (prefiltered by score margin)
    replicating the reference's exact fp32 arithmetic ((A+B) - 2M with
    IEEE fp32 rounding, first-index tie-break), which reproduces
    jnp.argmin's decisions bit-for-bit.  Rare tokens with uncertain
    candidate coverage fall back to an exact full-row re-score.

The argmin of the reference is numerically delicate: d sits on a ~256 base
so fp32 rounding quantizes scores into ~3e-5 buckets and ~1% of tokens are
decided by tie-breaking.  Replicating the fp32 rounding on the candidate
set is what makes the output match bit-for-bit.
"""

import numpy as np

K = 8192
D = 256
B = 16
HW = 1024            # 32*32
N = B * HW           # 16384 tokens
NCORES = 8
BPC = B // NCORES    # batches per core = 2
TPC = BPC * HW       # tokens per core = 2048
NBLK = TPC // 128    # 16 token blocks of 128
NTILE = 512          # matmul moving free dim
PSW = 1024           # psum tile width (2 banks)
ECH = 2048           # codebook SBUF chunk width
NG = 512             # groups per token after fold chain
GRP = K // NG        # 16 codes per group, stride NG

_CACHE = {}


def _build_nc():
    import concourse.tile as tile
    from concourse import bacc, mybir

    f32 = mybir.dt.float32
    bf16 = mybir.dt.bfloat16
    u16 = mybir.dt.uint16
    MAX = mybir.AluOpType.max

    nc = bacc.Bacc("TRN2", target_bir_lowering=False, debug=False,
                   num_devices=NCORES)
    z_d = nc.dram_tensor("zb", (BPC, D, HW), bf16, kind="ExternalInput").ap()
    e_d = nc.dram_tensor("eb", (D, K), bf16, kind="ExternalInput").ap()
    idx_d = nc.dram_tensor("idx8", (NBLK, 128, 8), u16, kind="ExternalOutput").ap()
    val_d = nc.dram_tensor("val8", (NBLK, 128, 8), bf16, kind="ExternalOutput").ap()

    with tile.TileContext(nc) as tc:
        with tc.tile_pool(name="w", bufs=1) as wpool, \
             tc.tile_pool(name="c", bufs=4) as cpool, \
             tc.tile_pool(name="f", bufs=3) as fpool, \
             tc.tile_pool(name="o", bufs=4) as opool, \
             tc.tile_pool(name="ps", bufs=4, space="PSUM") as pspool:
            # z: one merged DMA per batch; tile cols = kc*1024 + t
            zsb = []
            for b in range(BPC):
                t = wpool.tile([128, 2 * HW], bf16, tag=f"z{b}", name=f"z{b}")
                nc.sync.dma_start(
                    t[:].rearrange("p (c t) -> p c t", c=2),
                    z_d[b].rearrange("(c p) t -> p c t", c=2))
                zsb.append(t)
            # codebook in 4 column-chunks per kc so compute starts after the
            # first chunk lands instead of after the full 4MB load
            NQ = K // ECH
            esb = [[None] * NQ for _ in range(2)]
            for q in range(NQ):
                for kc in range(2):
                    t = wpool.tile([128, ECH], bf16, tag=f"e{kc}_{q}",
                                   name=f"e{kc}_{q}")
                    nc.sync.dma_start(
                        t[:], e_d[kc * 128:(kc + 1) * 128,
                                  q * ECH:(q + 1) * ECH])
                    esb[kc][q] = t

            def finish_block(st):
                """Tail folds + top-8 for a block (deferred one block)."""
                tb, ft, fa = st
                g0 = fpool.tile([128, 1024], bf16, name="g0")
                nc.vector.tensor_tensor(g0[:], ft[:, 0:1024],
                                        ft[:, 1024:2048], op=MAX)
                fb = fpool.tile([128, 1024], bf16, name="fb")
                nc.vector.tensor_tensor(fb[:], fa[:, 0:1024],
                                        fa[:, 1024:2048], op=MAX)
                fc = fpool.tile([128, 1024], bf16, name="fc")
                nc.vector.tensor_tensor(fc[:], g0[:], fb[:], op=MAX)
                fd = fpool.tile([128, NG], bf16, name="fd")
                nc.vector.tensor_tensor(fd[:], fc[:, 0:NG],
                                        fc[:, NG:2 * NG], op=MAX)
                v8 = opool.tile([128, 8], bf16, tag="v8", name="v8")
                i8 = opool.tile([128, 8], u16, tag="i8", name="i8")
                nc.vector.max(v8[:], fd[:])
                nc.vector.max_index(i8[:], v8[:], fd[:])
                nc.sync.dma_start(val_d[tb], v8[:])
                nc.sync.dma_start(idx_d[tb], i8[:])

            pending = None
            for tb in range(NBLK):
                b, tl = tb // 8, tb % 8
                lhs0 = zsb[b][:, tl * 128:(tl + 1) * 128]
                lhs1 = zsb[b][:, HW + tl * 128:HW + (tl + 1) * 128]
                # psum tile pt holds codes [1024*pt, 1024*(pt+1)).
                # pt 2,3 are DVE-consumed (fused fold, partnered with the
                # first-drained pt 0,1); the rest go ACT -> cblk at column
                # order [pt0|pt1|pt4|pt5|pt6|pt7]
                cblk = cpool.tile([128, 6 * PSW], bf16, name="cblk")
                ccol = {0: 0, 1: 1, 4: 2, 5: 3, 6: 4, 7: 5}
                t23 = []
                for pt in range(K // PSW):
                    ps = pspool.tile([128, PSW], f32, name="ps")
                    for h in range(PSW // NTILE):
                        nb = pt * (PSW // NTILE) + h
                        sl = slice(h * NTILE, (h + 1) * NTILE)
                        q, r = nb // 4, nb % 4
                        el = slice(r * NTILE, (r + 1) * NTILE)
                        nc.tensor.matmul(ps[:, sl], lhs0, esb[0][q][:, el],
                                         start=True, stop=False)
                        nc.tensor.matmul(ps[:, sl], lhs1, esb[1][q][:, el],
                                         start=False, stop=True)
                    if pt in (2, 3):
                        t23.append(ps)
                    else:
                        cc = ccol[pt]
                        nc.scalar.copy(cblk[:, cc * PSW:(cc + 1) * PSW], ps[:])
                # fold chain to groups {c + 512*m, m=0..15} (exact bf16
                # maxes; every pairing joins columns equal mod 512, so the
                # final groups are the full residue classes)
                ft = fpool.tile([128, 2048], bf16, name="ft")
                nc.vector.tensor_tensor(ft[:, 0:1024], cblk[:, 0:1024],
                                        t23[0][:], op=MAX)      # {c,2048+c}
                nc.vector.tensor_tensor(ft[:, 1024:2048], cblk[:, 1024:2048],
                                        t23[1][:], op=MAX)      # {1024+c,3072+c}
                fa = fpool.tile([128, 2048], bf16, name="fa")
                nc.vector.tensor_tensor(fa[:, 0:1024], cblk[:, 2048:3072],
                                        cblk[:, 4096:5120], op=MAX)
                nc.vector.tensor_tensor(fa[:, 1024:2048], cblk[:, 3072:4096],
                                        cblk[:, 5120:6144], op=MAX)
                if pending is not None:
                    finish_block(pending)
                pending = (tb, ft, fa)
            finish_block(pending)

    nc.compile()
    return nc


def _get_nc():
    if "nc" not in _CACHE:
        _CACHE["nc"] = _build_nc()
    return _CACHE["nc"]


def _run_device(z, e, trace=False):
    """Returns (group_idx (N,8) int64, group_val (N,8) float32)."""
    import ml_dtypes
    from concourse.bass_utils import run_bass_kernel_spmd

    nc = _get_nc()
    e2t = np.ascontiguousarray((2.0 * e).T).astype(ml_dtypes.bfloat16)
    zr = z.reshape(B, D, HW)
    in_maps = []
    for c in range(NCORES):
        zb = np.ascontiguousarray(zr[BPC * c:BPC * (c + 1)]).astype(
            ml_dtypes.bfloat16)
        in_maps.append({"zb": zb, "eb": e2t})
    try:
        res = run_bass_kernel_spmd(nc, in_maps, core_ids=list(range(NCORES)),
                                   trace=trace)
    except Exception:
        # transient NRT/device hiccups happen; one retry
        res = run_bass_kernel_spmd(nc, in_maps, core_ids=list(range(NCORES)),
                                   trace=trace)
    idx = np.concatenate(
        [res.results[c]["idx8"].reshape(TPC, 8).astype(np.int64)
         for c in range(NCORES)], axis=0)
    val = np.concatenate(
        [res.results[c]["val8"].reshape(TPC, 8).astype(np.float32)
         for c in range(NCORES)], axis=0)
    if trace:
        return idx, val, res
    return idx, val


def kernel(z, embedding):
    z = np.asarray(z, dtype=np.float32)
    e = np.asarray(embedding, dtype=np.float32)

    gidx, gval = _run_device(z, e)

    # Host: exact fp32 replication of the reference scoring on candidates.
    zp = np.ascontiguousarray(
        z.reshape(B, D, HW).transpose(0, 2, 1)).reshape(N, D)   # (N, D) f32
    z64 = zp.astype(np.float64)
    e64 = e.astype(np.float64)
    A32 = np.einsum("nd,nd->n", z64, z64).astype(np.float32)    # ||z_t||^2
    B32 = np.einsum("kd,kd->k", e64, e64).astype(np.float32)    # ||e_k||^2

    bad = (gidx < 0) | (gidx >= NG)
    gidx_s = np.where(bad, 0, gidx)

    # The winner's group-max is within (fp32 tie buckets + bf16 roundings +
    # bf16 matmul error) of the best group; only groups inside that margin
    # can contain the argmin.  Groups come sorted by descending score.
    MARGIN = 2.2e-4
    need = gval >= (gval[:, :1] - MARGIN)        # prefix mask, col 0 True
    nneed = need.sum(axis=1)

    idxf = np.zeros(N, dtype=np.int64)
    dmin = np.zeros(N, dtype=np.float32)
    dupw = np.zeros(N, dtype=bool)
    moff = NG * np.arange(GRP)
    for n in range(1, 9):
        tsel = np.nonzero(nneed == n)[0]
        if len(tsel) == 0:
            continue
        cand = (gidx_s[tsel, :n, None] + moff[None, None, :]).reshape(
            len(tsel), n * GRP)
        M32 = np.einsum("nd,njd->nj", z64[tsel], e64[cand],
                        optimize=True).astype(np.float32)
        Tj = A32[tsel, None] + B32[cand]         # fl32(A+B)
        dj = Tj - 2.0 * M32                      # fl32(T - C)
        dm = dj.min(axis=1)
        idxf[tsel] = np.where(dj == dm[:, None], cand, K + 1).min(axis=1)
        dmin[tsel] = dm

    # Fallback for tokens whose candidate set may not cover the tie zone:
    # top-8 group scores too crowded, or invalid device output.
    risky = (gval[:, 0] - gval[:, 7] < 2.5e-4) | bad.any(axis=1) | dupw
    if risky.any():
        rt = np.nonzero(risky)[0]
        Mrow = (z64[rt] @ e64.T).astype(np.float32)     # (R, K)
        drow = (A32[rt, None] + B32[None, :]) - 2.0 * Mrow
        idxf[rt] = np.argmin(drow, axis=1)

    z_q = e[idxf]                                        # (N, D) f32 exact rows
    diff = z_q.astype(np.float64) - z64
    m32 = np.float32(np.mean(diff * diff))
    q_loss = np.float32(m32 + np.float32(0.25) * m32)

    # straight-through estimator, replicated in fp32: out = zp + (z_q - zp)
    st = zp + (z_q - zp)                                 # all f32, IEEE RN
    out = np.ascontiguousarray(
        st.reshape(B, HW, D).transpose(0, 2, 1)).reshape(B, D, 32, 32)
    return out, q_loss
